# revision 7
# baseline (speedup 1.0000x reference)
"""Chamfer distance kernel for 8 Trainium2 NeuronCores.

Problem: pred/target (4, 8192, 3) fp32 -> scalar mean chamfer distance
(bidirectional nearest-neighbor squared distances, mean over batch).

Sharding (data parallel on batch x pred-half): core c handles batch
b = c // 2 and pred-half h = c % 2 (4096 of the 8192 pred points) against
ALL 8192 targets of that batch. Forward mins (over targets) complete per
core; backward row-mins (over preds) are per-half partials that the host
min-combines across the core pair.

Device math per core:
  d2[m, n] = ||q_m||^2 + ||p_n||^2 - 2 q_m . p_n   (m target, n pred)
computed as one K=13 matmul using bf16 hi/lo splitting (~2^-18 relative
error, and bf16 streams at 1 PE cycle/row where fp32 needs 4):
    Q_aug rows: [qh0..2, qh0..2, ql0..2, q2h, q2l, 1, 1]  (q* = split(-2q))
    P_aug rows: [ph0..2, pl0..2, ph0..2, 1,  1,  p2h, p2l]
  dot = qh.ph + qh.pl + ql.ph + q2 + p2 ~= -2 q.p + ||q||^2 + ||p||^2.

Tiling: out[m-chunk of 128 on partitions, n-chunk of 512 free] in PSUM;
4 banks (2048 free) per reduction group, double buffered (8 banks total).
Per group, two DVE instructions consume the fresh d2 values:
  1) tensor_reduce(min, free axis) on psum alone -> r[:, g*64+mi]
     (clean backward row-min for these 128 targets over this n-group)
  2) tensor_tensor_reduce: A[:, g] = min(psum, A[:, g]) elementwise
     (forward min accumulate; the fused reduce output is a running min
     contaminated by A and is discarded into a scratch slot).
Clamping to zero (reference's maximum(d2, 0)) commutes with min, so it is
applied on the host after all mins.

Outputs per core: A [128, 4096] fp32 (forward; partition-min left to the
host) and r [128, 128] fp32 (backward row-mins per (g, mi)). The host does
the partition mins, relu clamp, cross-core/pair combines and means in
float64, returning the fp32 scalar.
"""

import functools

import numpy as np
import ml_dtypes

import concourse.bacc as bacc
import concourse.mybir as mybir
import concourse.tile as tile
from concourse.bass_utils import run_bass_kernel_spmd

BF16 = ml_dtypes.bfloat16

B = 4            # batches
N = 8192         # points per cloud
NCORES = 8
NH = N // 2      # preds per core (4096)
K = 13           # augmented contraction dim
MI = N // 128    # 64 target chunks of 128
GF = 2048        # free elements per reduction group (4 psum banks)
NG = NH // GF    # 2 groups per mi
BIG = 3.0e38


def _split_bf16(x):
    """fp32 -> (hi, lo) bf16 pair with x ~= hi + lo (error ~2^-18 |x|)."""
    xh = x.astype(BF16)
    xl = (x - xh.astype(np.float32)).astype(BF16)
    return xh, xl


def _aug_inputs(pred, target):
    """Per-core augmented bf16 matrices: {"q_aug": [13, 8192], "p_aug": [13, 4096]}."""
    in_maps = []
    for c in range(NCORES):
        b, h = divmod(c, 2)
        q = np.asarray(target[b], dtype=np.float32)              # (8192, 3)
        p = np.asarray(pred[b][h * NH:(h + 1) * NH], dtype=np.float32)

        qh, ql = _split_bf16(-2.0 * q)
        q2h, q2l = _split_bf16(np.sum(q * q, axis=-1, dtype=np.float32))
        onesq = np.ones(N, dtype=BF16)
        q_aug = np.stack([
            qh[:, 0], qh[:, 1], qh[:, 2],
            qh[:, 0], qh[:, 1], qh[:, 2],
            ql[:, 0], ql[:, 1], ql[:, 2],
            q2h, q2l, onesq, onesq,
        ])                                                       # (13, 8192)

        ph, pl = _split_bf16(p)
        p2h, p2l = _split_bf16(np.sum(p * p, axis=-1, dtype=np.float32))
        onesp = np.ones(NH, dtype=BF16)
        p_aug = np.stack([
            ph[:, 0], ph[:, 1], ph[:, 2],
            pl[:, 0], pl[:, 1], pl[:, 2],
            ph[:, 0], ph[:, 1], ph[:, 2],
            onesp, onesp, p2h, p2l,
        ])                                                       # (13, 4096)
        in_maps.append({"q_aug": np.ascontiguousarray(q_aug),
                        "p_aug": np.ascontiguousarray(p_aug)})
    return in_maps


@functools.lru_cache(maxsize=1)
def _build_program():
    nc = bacc.Bacc("TRN2", target_bir_lowering=False, debug=False,
                   num_devices=NCORES)
    f32 = mybir.dt.float32
    bf16 = mybir.dt.bfloat16
    mn = mybir.AluOpType.min

    q_dram = nc.dram_tensor("q_aug", [K, N], bf16, kind="ExternalInput")
    p_dram = nc.dram_tensor("p_aug", [K, NH], bf16, kind="ExternalInput")
    a_dram = nc.dram_tensor("a_out", [128, NH], f32, kind="ExternalOutput")
    r_dram = nc.dram_tensor("r_out", [128, NG * MI], f32, kind="ExternalOutput")

    with tile.TileContext(nc) as tc:
        with tc.tile_pool(name="const", bufs=1) as cpool, \
             tc.tile_pool(name="psum", bufs=2, space="PSUM") as ppool:
            q_sb = cpool.tile([K, N], bf16)
            p_sb = cpool.tile([K, NH], bf16)
            a_sb = cpool.tile([128, NH], f32)
            r_sb = cpool.tile([128, NG * MI], f32)

            nc.sync.dma_start(out=q_sb[:], in_=q_dram.ap())
            nc.sync.dma_start(out=p_sb[:], in_=p_dram.ap())
            nc.gpsimd.memset(a_sb[:], BIG)

            for mi in range(MI):
                for g in range(NG):
                    ps = ppool.tile([128, GF], f32, tag="ps")
                    for j in range(GF // 512):
                        nj = (GF // 512) * g + j
                        nc.tensor.matmul(
                            ps[:, j * 512:(j + 1) * 512],
                            q_sb[:, mi * 128:(mi + 1) * 128],
                            p_sb[:, nj * 512:(nj + 1) * 512],
                            start=True, stop=True,
                        )
                    # clean backward row-min over this psum group alone
                    nc.vector.tensor_reduce(
                        out=r_sb[:, g * MI + mi: g * MI + mi + 1],
                        in_=ps[:],
                        axis=mybir.AxisListType.X,
                        op=mn,
                    )
                    # forward elementwise min-accumulate (in place)
                    nc.vector.tensor_tensor(
                        out=a_sb[:, g * GF:(g + 1) * GF],
                        in0=ps[:],
                        in1=a_sb[:, g * GF:(g + 1) * GF],
                        op=mn,
                    )

            nc.sync.dma_start(out=a_dram.ap(), in_=a_sb[:])
            nc.sync.dma_start(out=r_dram.ap(), in_=r_sb[:])

    nc.compile()
    return nc


def _host_reduce(results):
    """Combine per-core outputs into the final scalar (float64 internally)."""
    chamfers = []
    for b in range(B):
        fs = []
        bvecs = []
        for h in range(2):
            res = results[2 * b + h]
            A = np.asarray(res["a_out"], dtype=np.float64)    # [128, 4096]
            R = np.asarray(res["r_out"], dtype=np.float64)    # [128, 128]
            fs.append(A.min(axis=0))                          # [4096]
            bp = np.minimum(R[:, :MI], R[:, MI:])             # [128, 64]
            bvecs.append(bp.T.reshape(N))                     # m = 128*mi + p
        f = np.maximum(np.concatenate(fs), 0.0)               # [8192] fwd mins
        bv = np.maximum(np.minimum(bvecs[0], bvecs[1]), 0.0)  # [8192] bwd mins
        chamfers.append(f.mean() + bv.mean())
    return np.float32(np.mean(chamfers))


def kernel(pred, target):
    pred = np.asarray(pred, dtype=np.float32)
    target = np.asarray(target, dtype=np.float32)
    in_maps = _aug_inputs(pred, target)
    nc = _build_program()
    res = run_bass_kernel_spmd(nc, in_maps, core_ids=list(range(NCORES)))
    return np.array(_host_reduce(res.results), dtype=np.float32)
